# revision 18
# baseline (speedup 1.0000x reference)
"""CRLoss (hard-negative triplet mining over a [B,B] similarity matrix) on 8 trn2 cores.

Sharding: rows of `similarity` split across 8 cores (1024 rows each; 8 row-tiles
of [128, 8192] fp16 per core).

Device computes UNMASKED per-row max (hardest value incl. same-label cols) and
per-partition column-max partials of the fp16 matrix. No labels on device: the
label mask only matters for the ~B/4096-per-row same-label columns, so the host
(a) computes each row/col's max over its own label group (tiny: sum of squared
group sizes ~ 3*B elements) and (b) for the handful of rows/cols where that
excluded max ties the device's unmasked max, recomputes the masked max exactly
from the fp16 matrix. All loss arithmetic stays f32 on host with the exact f32
diagonal.

Why fp16 (not int16 as before): the DVE's 2x_1p perf mode only engages for
16-bit float dtypes, halving tensor_tensor cycles. Row maxes per tile use a
tensor_max fold tree at 2x (8192 -> 512) plus one 1x tensor_reduce on the 512
remainder (~4.6k cycles vs 8.25k for a plain reduce; InstTensorTensorReduce
and the custom-DVE ISA ops do not codegen in this walrus build - "ISA wrong
length"). Column partials use tensor_max chains at 2x, with the last tile done
in four column quarters so its results stream out over two DMA rings while the
final row-reduce still runs.

Sync discipline (this compiler build encodes ONE sync-wait per instruction):
an "absorber" 1-cell self-copy observes each chunk-DMA semaphore on DVE before
the chunk's first real consumer; output DMAs are observed by DVE memsets of a
garbage column each output DMA also reads (pure WAR); the kernel-tail drain
then only needs the DVE semaphore (_fix_tail_drain strips the rest).
"""

import os

import numpy as np

B = 8192
N_CORES = 8
ROWS_PER_CORE = B // N_CORES  # 1024
P = 128
N_TILES = ROWS_PER_CORE // P  # 8
H = B // 2  # 4096
Q = B // 4  # 2048
F = 512  # row fold-tree stops here; tensor_reduce finishes

_cache: dict = {}
last_results = None  # BassKernelResults from the most recent run (for test.py)


def _build_bass():
    import concourse.bass as bass
    import concourse.mybir as mybir
    from concourse.tile import TileContext

    f16 = mybir.dt.float16
    Alu = mybir.AluOpType
    nc = bass.Bass(target_bir_lowering=False)

    sim = nc.dram_tensor("sim", [N_TILES, P, B], f16, kind="ExternalInput")
    an = nc.dram_tensor("an", [P, N_TILES], f16, kind="ExternalOutput")
    cm = nc.dram_tensor("cm", [P, B], f16, kind="ExternalOutput")
    # The Tile scheduler has 8 DMA-completion semaphore lanes; a 9th DMA
    # reuses a lane and needs a second sync-wait, which this walrus build
    # rejects. Keep total DMA count at 8: 6 loads + colmax + an. Tile 0 is
    # loaded as two 1MB halves so compute starts ~4us sooner.
    CHUNKS = [(1, 2), (2, 4), (4, 6), (6, 8)]

    with TileContext(nc) as tc:
        with tc.tile_pool(name="pp", bufs=1) as pp:
            sa = pp.tile([P, N_TILES * B], f16, tag="simall")
            acc = pp.tile([P, B], f16, tag="acc")
            an_t = pp.tile([P, N_TILES], f16, tag="an")
            fold = pp.tile([P, H], f16, tag="fold")

            nc.sync.dma_start(out=sa[:, :H], in_=sim[0, :, :H])
            nc.sync.dma_start(out=sa[:, H:B], in_=sim[0, :, H:])
            for lo_t, hi_t in CHUNKS:
                nc.sync.dma_start(
                    out=sa[:, lo_t * B : hi_t * B].rearrange(
                        "p (t j) -> p t j", j=B
                    ),
                    in_=sim[lo_t:hi_t].rearrange("t p j -> p t j"),
                )

            def row_max(t):
                # tensor_max fold tree at 2x: 8192 -> 512, then one 1x
                # tensor_reduce over the 512 remainder.
                raw = sa[:, t * B : (t + 1) * B]
                nc.vector.tensor_max(fold[:, :H], raw[:, :H], raw[:, H:])
                w = H // 2
                while w >= F:
                    nc.vector.tensor_max(fold[:, :w], fold[:, :w], fold[:, w : 2 * w])
                    w //= 2
                nc.vector.tensor_reduce(
                    an_t[:, t : t + 1],
                    fold[:, : 2 * F][:, :F],
                    mybir.AxisListType.X,
                    Alu.max,
                )

            chunk_first = {lo_t for lo_t, _ in CHUNKS}
            last = N_TILES - 1
            for t in range(N_TILES):
                raw = sa[:, t * B : (t + 1) * B]
                if t == 0:
                    # Tile 0 arrives as two half-tile DMAs; fold each half on
                    # arrival (split tree costs the same as the plain tree).
                    nc.vector.tensor_copy(raw[:, :1], raw[:, :1])  # absorber L
                    nc.vector.tensor_max(
                        fold[:, :Q], raw[:, :Q], raw[:, Q:H]
                    )
                    nc.vector.tensor_copy(raw[:, H : H + 1], raw[:, H : H + 1])
                    nc.vector.tensor_max(
                        fold[:, Q:H], raw[:, H : H + Q], raw[:, H + Q :]
                    )
                    nc.vector.tensor_max(fold[:, :Q], fold[:, :Q], fold[:, Q:H])
                    w = Q // 2
                    while w >= F:
                        nc.vector.tensor_max(
                            fold[:, :w], fold[:, :w], fold[:, w : 2 * w]
                        )
                        w //= 2
                    nc.vector.tensor_reduce(
                        an_t[:, :1], fold[:, :F], mybir.AxisListType.X, Alu.max
                    )
                    continue
                if t in chunk_first:
                    # Absorber: observe the chunk's DMA semaphore on DVE so
                    # real consumers only carry the DVE-semaphore wait.
                    nc.vector.tensor_copy(raw[:, :1], raw[:, :1])
                if t != last:
                    row_max(t)
                    if t == 1:
                        nc.vector.tensor_max(acc[:], sa[:, :B], raw)
                    else:
                        nc.vector.tensor_max(acc[:], acc[:], raw)
                else:
                    # Last tile: column halves first, then one colmax DMA
                    # streams while the final row-reduce runs.
                    for q in range(2):
                        lo, hi = q * H, (q + 1) * H
                        nc.vector.tensor_max(
                            acc[:, lo:hi], acc[:, lo:hi], raw[:, lo:hi]
                        )
                    nc.scalar.dma_start(out=cm[:], in_=acc[:])
                    row_max(t)
                    nc.scalar.dma_start(out=an[:], in_=an_t[:])

            # Observe each out-DMA's semaphore on DVE by overwriting one cell
            # the DMA read (pure WAR dependency: one wait each).
            nc.vector.memset(acc[:, :1], 0)
            nc.vector.memset(an_t[:, :1], 0)

    _fix_tail_drain(nc)
    return nc


def _fix_tail_drain(nc):
    """This walrus build encodes a single sync-wait per instruction, but the
    kernel-tail drain waits on every DMA semaphore plus the DVE semaphore.
    Every DMA semaphore is observed by a DVE instruction (absorber copies for
    loads, garbage-column memsets for stores), so the DVE-semaphore wait alone
    transitively implies all of them: drop the rest."""
    dma_sems = set()
    for ins in nc.inst_map.values():
        if type(ins).__name__ == "InstDMACopy":
            si = getattr(ins, "sync_info", None)
            for u in (getattr(si, "on_update", None) or []):
                dma_sems.add(u.id)
    for ins in nc.inst_map.values():
        if type(ins).__name__ == "InstDrain":
            si = getattr(ins, "sync_info", None)
            w = (getattr(si, "on_wait", None) or []) if si else []
            if len(w) > 1:
                keep = [x for x in w if x.id not in dma_sems]
                assert len(keep) == 1, [(x.id, x.wait_value) for x in w]
                si.on_wait = keep


def _label_group_maxes(sim16f, lab):
    """For every row i: max over columns with the same label (incl. diagonal);
    for every column j: max over rows with the same label. O(sum |group|^2)."""
    erow = np.full(B, -np.inf, dtype=np.float32)
    ecol = np.full(B, -np.inf, dtype=np.float32)
    order = np.argsort(lab, kind="stable")
    sl = lab[order]
    starts = np.flatnonzero(np.r_[True, sl[1:] != sl[:-1]])
    bounds = np.r_[starts, len(sl)]
    groups = []
    for k in range(len(starts)):
        M = order[bounds[k] : bounds[k + 1]]
        sub = sim16f[np.ix_(M, M)]
        erow[M] = sub.max(axis=1)
        ecol[M] = sub.max(axis=0)
        groups.append(M)
    # member list per row index
    members = {}
    for M in groups:
        for i in M:
            members[int(i)] = M
    return erow, ecol, members


def kernel(similarity, labels, margin, semi):
    global last_results
    from concourse.bass_utils import run_bass_kernel_spmd

    sim = np.asarray(similarity, dtype=np.float32)
    lab = np.asarray(labels).reshape(-1)
    marg = np.asarray(margin, dtype=np.float32).reshape(-1)

    sim16 = sim.astype(np.float16)

    if "nc" not in _cache:
        _cache["nc"] = _build_bass()
    nc = _cache["nc"]

    in_maps = []
    for c in range(N_CORES):
        r0 = c * ROWS_PER_CORE
        in_maps.append(
            {"sim": sim16[r0 : r0 + ROWS_PER_CORE].reshape(N_TILES, P, B)}
        )

    trace = os.environ.get("CRL_TRACE", "0") == "1"
    res = run_bass_kernel_spmd(
        nc, in_maps, core_ids=list(range(N_CORES)), trace=trace
    )
    last_results = res

    # Device unmasked maxes. Row r = c*1024 + t*128 + p lives at an[p, t].
    rmax = np.concatenate(
        [r["an"][:, :N_TILES].astype(np.float32).T.reshape(-1) for r in res.results]
    )  # [B]
    part = np.stack([r["cm"] for r in res.results])  # [8,128,B]
    cmax = part.astype(np.float32).max(axis=(0, 1))  # [B]

    # Host-side label-mask fixup.
    sim16f = sim16.astype(np.float32)
    erow, ecol, members = _label_group_maxes(sim16f, lab)

    an_row = rmax.copy()
    for i in np.flatnonzero(erow >= rmax):
        r = sim16f[i].copy()
        r[members[int(i)]] = -np.inf
        an_row[i] = r.max()
    an_col = cmax.copy()
    for j in np.flatnonzero(ecol >= cmax):
        c = sim16f[:, j].copy()
        c[members[int(j)]] = -np.inf
        an_col[j] = c.max()

    ap = np.ascontiguousarray(np.diagonal(sim))  # exact f32
    mam = marg - ap

    def one_side(an):
        valid = an > ap
        loss = np.maximum(mam + an, np.float32(0.0))
        return np.where(valid, loss, np.float32(0.0)).sum(dtype=np.float32)

    total = np.float32(one_side(an_row)) + np.float32(one_side(an_col))
    return np.asarray(total, dtype=np.float32)
